# revision 13
# baseline (speedup 1.0000x reference)
"""Trainium2 Bass kernel for nn_Aggregate (segment_reduce).

Reference computation:
    cat_idx = idx_inputs[:, argmax(softmax(cat_mask))]          # [N]
    agg     = segment_sum(inputs[:, 16:], cat_idx, 100000)       # [S, 128]
    out     = agg[cat_idx][:, top32(softmax(numeric_mask))] * conf

Strategy (v2 — unsegmented coarse scan + lattice extraction):
  * Only the 32 top-k numeric columns survive to the output, and segment_sum
    is linear per column -> select those 32 columns FIRST (4x less data)
    and fold the conf scaling into them.
  * Sort rows by segment on the host.  Each segment is then one contiguous
    run; its sum is S[end] - S[start-1] where S is a plain (unsegmented)
    per-stream prefix sum -- no masks on the device at all.  The scan state
    and the boundary differences are fp32 on-device, so there is no
    catastrophic cancellation; only the final output is rounded (fp16).
  * 4x coarsening: each run is padded to a multiple of 4 rows and split
    into 4 "phase" planes (rows 4c+0..4c+3).  The stock tensor_tensor_scan
    runs at ~2 cycles/column, so scanning at 1/4 resolution is the win.
    The 4->2 phase reduction is split across engines: one bf16 2x-mode
    tensor_add on the DVE (p0+=p1), one on the otherwise-idle GPSIMD
    (p2+=p3), and the final 2->1 add is FREE: the scan's recurrence is
    state = (data0 + state) + data1, so it consumes both partial streams
    directly.
  * Output compaction: runs are bucketed by coarse length l=ceil(len/4)
    and dealt uniformly to 8 cores x 4 partition-groups (dummy runs pad
    each bucket to a multiple of 32), so within a bucket the run-end
    prefix values form a regular lattice of stride l.  One strided
    tensor_sub per bucket computes all its segment sums (S[end]-S[end-l])
    straight into a compact fp16 output tile: ~0.8 MB leaves each core
    instead of the full 8 MB cumsum.
  * Host does only routing (sort, bucket, deal, gather) - every add that
    touches row data happens on the device.

Everything data-dependent (bucket geometry, lattice offsets) is baked into
the compiled graph; build_bass() therefore runs after prepare().
"""

import sys
import types

import ml_dtypes
import numpy as np

if "/opt/trn_rl_repo" not in sys.path:
    sys.path.insert(0, "/opt/trn_rl_repo")

import concourse.bacc as bacc
import concourse.mybir as mybir
import concourse.tile as tile
import concourse.dve_ops as dve_ops
from concourse.dve_spec import C0, Spec, Src0, Src1
from concourse.dve_spec import AluOp as DveAluOp
from concourse.dve_spec import scan as dve_scan


def _register_pair_add_scan():
    """Custom DVE op: out[k] = s0 + sum_{j<=k} (in0[j] + in1[j]).

    The stock tensor_tensor_scan routes its state feedback backward
    through the 8-stage pipe and runs at ~2 cycles/element; this Spec's
    scan() combine reads CURR_ALU_OUT (a one-cycle recurrence, no
    bubble) and additionally fuses the final pair-add of the two phase
    streams.  The per-NEFF uop table ships via the standard
    ant.dve_table HLO frontend-attribute path."""
    name = "PAIR_ADD_SCAN_AGG"
    for op in dve_ops.OPS:
        if op.name == name:
            return op
    spec = Spec(
        body=dve_scan(DveAluOp.ADD, Src0 + Src1, init=C0),
        reference=lambda in0, in1, s0, s1, imm2: (
            np.cumsum(in0.astype(np.float32) + in1.astype(np.float32),
                      axis=-1) + np.asarray(s0, dtype=np.float32)),
    )
    op = dve_ops.DveOp(
        name, spec, subdim=False,
        uops_sha={"v3": "8b49596cd428b415", "v4": "9f3b8a1ce4265eb2"},
    )
    dve_ops.OPS.append(op)
    dve_ops.CUSTOM_DVE_SPECS[name] = spec
    dve_ops._SUB_OPCODE_FOR_NAME[name] = (
        max(dve_ops._SUB_OPCODE_FOR_NAME.values()) + 1)
    return op


_PAIR_ADD_SCAN = _register_pair_add_scan()

# ----------------------------------------------------------------------------
# problem constants (hardcoded per spec)
N_ROWS = 1_000_000
NUM_CAT = 16
NUM_NUMERICS = 128
N_ARY = 32
NUM_SEGMENTS = 100_000

NCORES = 8
GROUPS = 4                    # partition-groups per core (32 feats each)
NSTREAM = NCORES * GROUPS     # 32 independent scan streams
PH = 4                        # coarsening factor == phase planes
NWIN = 4                      # pipeline windows per core

BF16 = ml_dtypes.bfloat16
F16 = np.float16

_dt = mybir.dt

_CACHE: dict = {}


def _ensure_axon_hooks():
    """bass_utils imports antenv.axon_hooks for trace=True; provide a shim
    so the import never fails (hook stays None unless a profiler sets it)."""
    if "antenv.axon_hooks" in sys.modules:
        return sys.modules["antenv.axon_hooks"]
    mod = types.ModuleType("antenv.axon_hooks")
    hook = [None]
    mod.set_axon_ntff_profile_hook = lambda h: hook.__setitem__(0, h)
    mod.get_axon_ntff_profile_hook = lambda: hook[0]
    sys.modules["antenv.axon_hooks"] = mod
    return mod


def _softmax64(v):
    v = np.asarray(v, dtype=np.float64)
    e = np.exp(v - v.max())
    return e / e.sum()


def prepare(inputs, idx_inputs, cat_mask, numeric_mask):
    """Host-side prep: top-k, column select + conf scale, sort, bucket by
    coarse run length, deal runs to 32 streams, build phase planes.

    Returns (in_maps, meta); also stashes the device-graph geometry in
    _CACHE["geo"] for build_bass().
    """
    cat_mask = np.asarray(cat_mask)
    numeric_mask = np.asarray(numeric_mask)
    cm = _softmax64(cat_mask)
    ti = int(np.argmax(cm))                     # top_k(1) -> first max
    top_cat_val = cm[ti]
    nm = _softmax64(numeric_mask)
    order = np.argsort(-nm, kind="stable")[:N_ARY]   # descending, ties->low idx
    conf = ((nm[order] + top_cat_val) / 2.0).astype(np.float32)

    seg = np.ascontiguousarray(np.asarray(idx_inputs)[:, ti]).astype(np.int32)
    perm = np.argsort(seg, kind="stable")
    seg_s = seg[perm]

    inputs = np.asarray(inputs)
    sel = inputs[:, NUM_CAT + order].astype(np.float32) * conf[None, :]
    xs = sel[perm].astype(BF16)                  # [N, 32] sorted rows, bf16

    # ---- run bookkeeping ----------------------------------------------
    isstart = np.empty(N_ROWS, dtype=bool)
    isstart[0] = True
    isstart[1:] = seg_s[1:] != seg_s[:-1]
    rank_s = np.cumsum(isstart) - 1              # [N] run index of each row
    start_pos = np.flatnonzero(isstart)          # [R]
    nruns = len(start_pos)
    lens = np.empty(nruns, dtype=np.int64)
    lens[:-1] = np.diff(start_pos)
    lens[-1] = N_ROWS - start_pos[-1]
    seg_of_run = seg_s[start_pos]                # [R]
    lp = (lens + PH - 1) // PH                   # coarse slot length per run

    # ---- bucket by coarse length, deal to 32 streams ------------------
    # stream s <-> (core = s // GROUPS, group = s % GROUPS)
    blens = np.unique(lp)
    s_of_run = np.empty(nruns, dtype=np.int64)
    k_of_run = np.empty(nruns, dtype=np.int64)   # slot index within bucket
    bkt_of_run = np.empty(nruns, dtype=np.int64)
    buckets = []                                 # (l, q, B, O) per bucket
    base = 1                                     # coarse col 0 = zero column
    out_base = 0
    for bi, l in enumerate(blens):
        ridx = np.flatnonzero(lp == l)
        m = len(ridx)
        q = -(-m // NSTREAM)                     # slots per stream
        # slot grid [q, NSTREAM]; run j -> (k = j // NSTREAM, s = j % NSTREAM)
        s_of_run[ridx] = np.arange(m) % NSTREAM
        k_of_run[ridx] = np.arange(m) // NSTREAM
        bkt_of_run[ridx] = bi
        buckets.append((int(l), int(q), int(base), int(out_base)))
        base += q * l
        out_base += q
    C4 = base
    Q = out_base
    Wc = -(-C4 // NWIN)
    Wc = (Wc + 7) // 8 * 8                       # round window to mult of 8
    C4pad = Wc * NWIN

    bucket_B = np.array([b[2] for b in buckets], dtype=np.int64)
    bucket_O = np.array([b[3] for b in buckets], dtype=np.int64)
    bucket_L = np.array([b[0] for b in buckets], dtype=np.int64)
    off_of_run = bucket_B[bkt_of_run] + k_of_run * bucket_L[bkt_of_run]
    outcol_of_run = bucket_O[bkt_of_run] + k_of_run

    # ---- scatter sorted rows into per-stream phase-resolved planes ----
    big = np.zeros((NSTREAM, C4pad * PH, N_ARY), dtype=BF16)
    within = np.arange(N_ROWS, dtype=np.int64) - start_pos[rank_s]
    srow = s_of_run[rank_s]
    posrow = PH * off_of_run[rank_s] + within
    big.reshape(-1, N_ARY)[srow * (C4pad * PH) + posrow] = xs

    # [NSTREAM, C4pad, PH, 32] -> [cores, 128parts, NWIN, PH, Wc], with the
    # phase planes ordered [p0, p2, p1, p3] so that ONE tensor_add of the
    # [p0|p2] half with the [p1|p3] half yields [a01|a23] in one instruction
    planes = big.reshape(NCORES, GROUPS, NWIN, Wc, PH, N_ARY)
    planes = planes.transpose(0, 1, 5, 2, 4, 3)  # [8, g, f, w, p, c]
    planes = np.ascontiguousarray(planes).reshape(NCORES, 128, NWIN, PH, Wc)
    planes = np.ascontiguousarray(planes[:, :, :, [0, 2, 1, 3], :])

    in_maps = []
    for i in range(NCORES):
        in_maps.append({"xin": planes[i].view(np.uint8)})

    _CACHE["geo"] = {"C4pad": C4pad, "Wc": Wc, "Q": Q, "buckets": buckets}
    meta = {
        "seg": seg,
        "seg_of_run": seg_of_run,
        "core_of_run": s_of_run // GROUPS,
        "group_of_run": s_of_run % GROUPS,
        "outcol_of_run": outcol_of_run,
        "Q": Q,
    }
    return in_maps, meta


def build_bass():
    """Build + compile the (SPMD, per-core identical) Bass graph.

    Geometry (window size, extraction lattice) comes from prepare()'s
    stash, so prepare() must run first.
    """
    if "nc" in _CACHE:
        return _CACHE["nc"]
    geo = _CACHE["geo"]
    C4pad, Wc, Q, buckets = geo["C4pad"], geo["Wc"], geo["Q"], geo["buckets"]

    nc = bacc.Bacc("TRN2", target_bir_lowering=False, debug=False,
                   num_devices=NCORES)
    xin = nc.dram_tensor("xin", [128, NWIN, PH, Wc * 2], _dt.uint8,
                         kind="ExternalInput").ap()
    xout = nc.dram_tensor("out", [128, Q * 2], _dt.uint8,
                          kind="ExternalOutput").ap()

    # last window whose scan a bucket's extraction lattice depends on
    def last_win(B, q, l):
        return min(NWIN - 1, (B + q * l - 1) // Wc)

    subs_after = {w: [] for w in range(NWIN)}
    for bkt in buckets:
        l, q, B, O = bkt
        subs_after[last_win(B, q, l)].append(bkt)

    with tile.TileContext(nc) as tc:
        with tc.tile_pool(name="pp", bufs=1) as pool, \
             tc.tile_pool(name="xp", bufs=3) as xpool, \
             tc.tile_pool(name="ap", bufs=2) as apool:
            S = pool.tile([128, C4pad], _dt.float32, tag="S")
            ot = pool.tile([128, Q], _dt.float16, tag="o")
            for w in range(NWIN):
                a, b = w * Wc, (w + 1) * Wc
                xt = xpool.tile([128, PH * Wc], _dt.bfloat16, tag="x",
                                name="xt")
                nc.sync.dma_start(
                    out=xt[:, 0:2 * Wc],
                    in_=xin[:, w, 0:2, :].bitcast(_dt.bfloat16))
                nc.scalar.dma_start(
                    out=xt[:, 2 * Wc:4 * Wc],
                    in_=xin[:, w, 2:4, :].bitcast(_dt.bfloat16))
                aa = apool.tile([128, 2 * Wc], _dt.bfloat16, tag="aa")
                nc.vector.tensor_add(out=aa[:], in0=xt[:, 0:2 * Wc],
                                     in1=xt[:, 2 * Wc:4 * Wc])
                init = 0.0 if w == 0 else S[:, a - 1:a]
                nc.vector._custom_dve(_PAIR_ADD_SCAN, out=S[:, a:b],
                                      in0=aa[:, 0:Wc], in1=aa[:, Wc:2 * Wc],
                                      s0=init)
                # strided lattice subtracts run on the otherwise-idle GPSIMD
                # as soon as the last covering scan is done
                for (l, q, B, O) in subs_after[w]:
                    e0 = B + l - 1
                    nc.gpsimd.tensor_sub(
                        out=ot[:, O:O + q],
                        in0=S[:, e0:e0 + q * l:l],
                        in1=S[:, B - 1:B - 1 + q * l:l])
            nc.scalar.dma_start(out=xout[:], in_=ot[:].bitcast(_dt.uint8))
    nc.compile()
    _CACHE["nc"] = nc
    return nc


def postprocess(results, meta):
    """Pull per-run sums from the compacted device outputs, expand to rows."""
    table = np.zeros((NUM_SEGMENTS, N_ARY), dtype=np.float32)
    core = meta["core_of_run"]
    group = meta["group_of_run"]
    outcol = meta["outcol_of_run"]
    for i in range(NCORES):
        O = results[i]["out"].view(F16).astype(np.float32)       # [128, Q]
        O = O.reshape(GROUPS, 32, meta["Q"])
        m = core == i
        table[meta["seg_of_run"][m]] = O[group[m], :, outcol[m]]
    return table[meta["seg"]]


def run(in_maps, trace=False, trace_kwargs=None):
    _ensure_axon_hooks()
    from concourse.bass_utils import run_bass_kernel_spmd
    nc = build_bass()
    return run_bass_kernel_spmd(nc, in_maps, core_ids=list(range(NCORES)),
                                trace=trace, **(trace_kwargs or {}))


def kernel(inputs, idx_inputs, cat_mask, numeric_mask):
    in_maps, meta = prepare(inputs, idx_inputs, cat_mask, numeric_mask)
    res = run(in_maps, trace=False)
    return postprocess(res.results, meta)


# revision 18
# speedup vs baseline: 1.0672x; 1.0672x over previous
"""Trainium2 Bass kernel for nn_Aggregate (segment_reduce).

Reference computation:
    cat_idx = idx_inputs[:, argmax(softmax(cat_mask))]          # [N]
    agg     = segment_sum(inputs[:, 16:], cat_idx, 100000)       # [S, 128]
    out     = agg[cat_idx][:, top32(softmax(numeric_mask))] * conf

Strategy (v2 — unsegmented coarse scan + lattice extraction):
  * Only the 32 top-k numeric columns survive to the output, and segment_sum
    is linear per column -> select those 32 columns FIRST (4x less data)
    and fold the conf scaling into them.
  * Sort rows by segment on the host.  Each segment is then one contiguous
    run; its sum is S[end] - S[start-1] where S is a plain (unsegmented)
    per-stream prefix sum -- no masks on the device at all.  The scan state
    and the boundary differences are fp32 on-device, so there is no
    catastrophic cancellation; only the final output is rounded (fp16).
  * 2x coarsening with ZERO extra instructions: each run is padded to an
    even number of rows and split into 2 "phase" planes (rows 2c, 2c+1).
    A custom DVE op (see _register_pair_add_scan) computes
    state += in0[c] + in1[c] at 1 column/cycle -- it consumes both phase
    streams directly, so the pair-add costs nothing and the scan runs at
    half resolution.  (The stock tensor_tensor_scan needs ~2 cyc/col;
    the custom op's scan() combine is a one-cycle recurrence.)
  * Output compaction: runs are bucketed by coarse length l=ceil(len/4)
    and dealt uniformly to 8 cores x 4 partition-groups (dummy runs pad
    each bucket to a multiple of 32), so within a bucket the run-end
    prefix values form a regular lattice of stride l.  One strided
    tensor_sub per bucket computes all its segment sums (S[end]-S[end-l])
    straight into a compact fp16 output tile: ~0.8 MB leaves each core
    instead of the full 8 MB cumsum.
  * Host does only routing (sort, bucket, deal, gather) - every add that
    touches row data happens on the device.

Everything data-dependent (bucket geometry, lattice offsets) is baked into
the compiled graph; build_bass() therefore runs after prepare().
"""

import sys
import types

import ml_dtypes
import numpy as np

if "/opt/trn_rl_repo" not in sys.path:
    sys.path.insert(0, "/opt/trn_rl_repo")

import concourse.bacc as bacc
import concourse.mybir as mybir
import concourse.tile as tile
import concourse.dve_ops as dve_ops
from concourse.dve_spec import C0, Spec, Src0, Src1
from concourse.dve_spec import AluOp as DveAluOp
from concourse.dve_spec import scan as dve_scan


def _register_pair_add_scan():
    """Custom DVE op: out[k] = s0 + sum_{j<=k} (in0[j] + in1[j]).

    The stock tensor_tensor_scan routes its state feedback backward
    through the 8-stage pipe and runs at ~2 cycles/element; this Spec's
    scan() combine reads CURR_ALU_OUT (a one-cycle recurrence, no
    bubble) and additionally fuses the final pair-add of the two phase
    streams.  The per-NEFF uop table ships via the standard
    ant.dve_table HLO frontend-attribute path."""
    name = "PAIR_ADD_SCAN_AGG"
    for op in dve_ops.OPS:
        if op.name == name:
            return op
    spec = Spec(
        body=dve_scan(DveAluOp.ADD, Src0 + Src1, init=C0),
        reference=lambda in0, in1, s0, s1, imm2: (
            np.cumsum(in0.astype(np.float32) + in1.astype(np.float32),
                      axis=-1) + np.asarray(s0, dtype=np.float32)),
    )
    op = dve_ops.DveOp(
        name, spec, subdim=False,
        uops_sha={"v3": "8b49596cd428b415", "v4": "9f3b8a1ce4265eb2"},
    )
    dve_ops.OPS.append(op)
    dve_ops.CUSTOM_DVE_SPECS[name] = spec
    dve_ops._SUB_OPCODE_FOR_NAME[name] = (
        max(dve_ops._SUB_OPCODE_FOR_NAME.values()) + 1)
    return op


_PAIR_ADD_SCAN = _register_pair_add_scan()

# ----------------------------------------------------------------------------
# problem constants (hardcoded per spec)
N_ROWS = 1_000_000
NUM_CAT = 16
NUM_NUMERICS = 128
N_ARY = 32
NUM_SEGMENTS = 100_000

NCORES = 8
GROUPS = 4                    # partition-groups per core (32 feats each)
NSTREAM = NCORES * GROUPS     # 32 independent scan streams
PH = 2                        # coarsening factor == phase planes
NWIN = 4                      # pipeline windows per core

BF16 = ml_dtypes.bfloat16
F16 = np.float16

_dt = mybir.dt

_CACHE: dict = {}


def _ensure_axon_hooks():
    """bass_utils imports antenv.axon_hooks for trace=True; provide a shim
    so the import never fails (hook stays None unless a profiler sets it)."""
    if "antenv.axon_hooks" in sys.modules:
        return sys.modules["antenv.axon_hooks"]
    mod = types.ModuleType("antenv.axon_hooks")
    hook = [None]
    mod.set_axon_ntff_profile_hook = lambda h: hook.__setitem__(0, h)
    mod.get_axon_ntff_profile_hook = lambda: hook[0]
    sys.modules["antenv.axon_hooks"] = mod
    return mod


def _softmax64(v):
    v = np.asarray(v, dtype=np.float64)
    e = np.exp(v - v.max())
    return e / e.sum()


def prepare(inputs, idx_inputs, cat_mask, numeric_mask):
    """Host-side prep: top-k, column select + conf scale, sort, bucket by
    coarse run length, deal runs to 32 streams, build phase planes.

    Returns (in_maps, meta); also stashes the device-graph geometry in
    _CACHE["geo"] for build_bass().
    """
    cat_mask = np.asarray(cat_mask)
    numeric_mask = np.asarray(numeric_mask)
    cm = _softmax64(cat_mask)
    ti = int(np.argmax(cm))                     # top_k(1) -> first max
    top_cat_val = cm[ti]
    nm = _softmax64(numeric_mask)
    order = np.argsort(-nm, kind="stable")[:N_ARY]   # descending, ties->low idx
    conf = ((nm[order] + top_cat_val) / 2.0).astype(np.float32)

    seg = np.ascontiguousarray(np.asarray(idx_inputs)[:, ti]).astype(np.int32)
    perm = np.argsort(seg, kind="stable")
    seg_s = seg[perm]

    inputs = np.asarray(inputs)
    sel = inputs[:, NUM_CAT + order].astype(np.float32) * conf[None, :]
    xs = sel[perm].astype(BF16)                  # [N, 32] sorted rows, bf16

    # ---- run bookkeeping ----------------------------------------------
    isstart = np.empty(N_ROWS, dtype=bool)
    isstart[0] = True
    isstart[1:] = seg_s[1:] != seg_s[:-1]
    rank_s = np.cumsum(isstart) - 1              # [N] run index of each row
    start_pos = np.flatnonzero(isstart)          # [R]
    nruns = len(start_pos)
    lens = np.empty(nruns, dtype=np.int64)
    lens[:-1] = np.diff(start_pos)
    lens[-1] = N_ROWS - start_pos[-1]
    seg_of_run = seg_s[start_pos]                # [R]
    lp = (lens + PH - 1) // PH                   # coarse slot length per run

    # ---- bucket by coarse length, deal to 32 streams ------------------
    # stream s <-> (core = s // GROUPS, group = s % GROUPS)
    blens = np.unique(lp)
    s_of_run = np.empty(nruns, dtype=np.int64)
    k_of_run = np.empty(nruns, dtype=np.int64)   # slot index within bucket
    bkt_of_run = np.empty(nruns, dtype=np.int64)
    buckets = []                                 # (l, q, B, O) per bucket
    base = 1                                     # coarse col 0 = zero column
    out_base = 0
    for bi, l in enumerate(blens):
        ridx = np.flatnonzero(lp == l)
        m = len(ridx)
        q = -(-m // NSTREAM)                     # slots per stream
        # slot grid [q, NSTREAM]; run j -> (k = j // NSTREAM, s = j % NSTREAM)
        s_of_run[ridx] = np.arange(m) % NSTREAM
        k_of_run[ridx] = np.arange(m) // NSTREAM
        bkt_of_run[ridx] = bi
        buckets.append((int(l), int(q), int(base), int(out_base)))
        base += q * l
        out_base += q
    C4 = base
    Q = out_base
    Wc = -(-C4 // NWIN)
    Wc = (Wc + 7) // 8 * 8                       # round window to mult of 8
    C4pad = Wc * NWIN

    bucket_B = np.array([b[2] for b in buckets], dtype=np.int64)
    bucket_O = np.array([b[3] for b in buckets], dtype=np.int64)
    bucket_L = np.array([b[0] for b in buckets], dtype=np.int64)
    off_of_run = bucket_B[bkt_of_run] + k_of_run * bucket_L[bkt_of_run]
    outcol_of_run = bucket_O[bkt_of_run] + k_of_run

    # ---- scatter sorted rows into per-stream phase-resolved planes ----
    big = np.zeros((NSTREAM, C4pad * PH, N_ARY), dtype=BF16)
    within = np.arange(N_ROWS, dtype=np.int64) - start_pos[rank_s]
    srow = s_of_run[rank_s]
    posrow = PH * off_of_run[rank_s] + within
    big.reshape(-1, N_ARY)[srow * (C4pad * PH) + posrow] = xs

    # [NSTREAM, C4pad, PH, 32] -> [cores, 128parts, NWIN, PH, Wc]
    planes = big.reshape(NCORES, GROUPS, NWIN, Wc, PH, N_ARY)
    planes = planes.transpose(0, 1, 5, 2, 4, 3)  # [8, g, f, w, p, c]
    planes = np.ascontiguousarray(planes).reshape(NCORES, 128, NWIN, PH, Wc)

    in_maps = []
    for i in range(NCORES):
        in_maps.append({"xin": planes[i].view(np.uint8)})

    _CACHE["geo"] = {"C4pad": C4pad, "Wc": Wc, "Q": Q, "buckets": buckets}
    meta = {
        "seg": seg,
        "seg_of_run": seg_of_run,
        "core_of_run": s_of_run // GROUPS,
        "group_of_run": s_of_run % GROUPS,
        "outcol_of_run": outcol_of_run,
        "Q": Q,
    }
    return in_maps, meta


def build_bass():
    """Build + compile the (SPMD, per-core identical) Bass graph.

    Geometry (window size, extraction lattice) comes from prepare()'s
    stash, so prepare() must run first.
    """
    if "nc" in _CACHE:
        return _CACHE["nc"]
    geo = _CACHE["geo"]
    C4pad, Wc, Q, buckets = geo["C4pad"], geo["Wc"], geo["Q"], geo["buckets"]

    nc = bacc.Bacc("TRN2", target_bir_lowering=False, debug=False,
                   num_devices=NCORES)
    xin = nc.dram_tensor("xin", [128, NWIN, PH, Wc * 2], _dt.uint8,
                         kind="ExternalInput").ap()
    xout = nc.dram_tensor("out", [128, Q * 2], _dt.uint8,
                          kind="ExternalOutput").ap()

    # last window whose scan a bucket's extraction lattice depends on
    def last_win(B, q, l):
        return min(NWIN - 1, (B + q * l - 1) // Wc)

    subs_after = {w: [] for w in range(NWIN)}
    for bkt in buckets:
        l, q, B, O = bkt
        subs_after[last_win(B, q, l)].append(bkt)

    with tile.TileContext(nc) as tc:
        with tc.tile_pool(name="pp", bufs=1) as pool, \
             tc.tile_pool(name="xp", bufs=3) as xpool:
            S = pool.tile([128, C4pad], _dt.float32, tag="S")
            ot = pool.tile([128, Q], _dt.float16, tag="o")
            for w in range(NWIN):
                a, b = w * Wc, (w + 1) * Wc
                xt = xpool.tile([128, PH * Wc], _dt.bfloat16, tag="x",
                                name="xt")
                nc.sync.dma_start(
                    out=xt[:, 0:Wc],
                    in_=xin[:, w, 0, :].bitcast(_dt.bfloat16))
                nc.scalar.dma_start(
                    out=xt[:, Wc:2 * Wc],
                    in_=xin[:, w, 1, :].bitcast(_dt.bfloat16))
                init = 0.0 if w == 0 else S[:, a - 1:a]
                nc.vector._custom_dve(_PAIR_ADD_SCAN, out=S[:, a:b],
                                      in0=xt[:, 0:Wc], in1=xt[:, Wc:2 * Wc],
                                      s0=init)
                # strided lattice subtracts run on the otherwise-idle GPSIMD
                # as soon as the last covering scan is done
                for (l, q, B, O) in subs_after[w]:
                    e0 = B + l - 1
                    nc.gpsimd.tensor_sub(
                        out=ot[:, O:O + q],
                        in0=S[:, e0:e0 + q * l:l],
                        in1=S[:, B - 1:B - 1 + q * l:l])
            nc.scalar.dma_start(out=xout[:], in_=ot[:].bitcast(_dt.uint8))
    nc.compile()
    _CACHE["nc"] = nc
    return nc


def postprocess(results, meta):
    """Pull per-run sums from the compacted device outputs, expand to rows."""
    table = np.zeros((NUM_SEGMENTS, N_ARY), dtype=np.float32)
    core = meta["core_of_run"]
    group = meta["group_of_run"]
    outcol = meta["outcol_of_run"]
    for i in range(NCORES):
        O = results[i]["out"].view(F16).astype(np.float32)       # [128, Q]
        O = O.reshape(GROUPS, 32, meta["Q"])
        m = core == i
        table[meta["seg_of_run"][m]] = O[group[m], :, outcol[m]]
    return table[meta["seg"]]


def run(in_maps, trace=False, trace_kwargs=None):
    _ensure_axon_hooks()
    from concourse.bass_utils import run_bass_kernel_spmd
    nc = build_bass()
    return run_bass_kernel_spmd(nc, in_maps, core_ids=list(range(NCORES)),
                                trace=trace, **(trace_kwargs or {}))


def kernel(inputs, idx_inputs, cat_mask, numeric_mask):
    in_maps, meta = prepare(inputs, idx_inputs, cat_mask, numeric_mask)
    res = run(in_maps, trace=False)
    return postprocess(res.results, meta)


# revision 20
# speedup vs baseline: 1.1585x; 1.0855x over previous
"""Trainium2 Bass kernel for nn_Aggregate (segment_reduce).

Reference computation:
    cat_idx = idx_inputs[:, argmax(softmax(cat_mask))]          # [N]
    agg     = segment_sum(inputs[:, 16:], cat_idx, 100000)       # [S, 128]
    out     = agg[cat_idx][:, top32(softmax(numeric_mask))] * conf

Strategy (v6 -- one custom paged-sum DVE op does everything):
  * Only the 32 top-k numeric columns survive to the output, and segment_sum
    is linear per column -> select those 32 columns FIRST (4x less data)
    and fold the conf scaling into them.
  * Sort rows by segment on the host.  Each segment is one contiguous run.
  * Runs are padded to an even number of rows (split into 2 interleaved
    "phase" planes), bucketed by half-length l = ceil(len/2), and dealt
    uniformly to 8 cores x 4 partition-groups (dummy runs pad each bucket
    to a multiple of 32), so each bucket region is a regular [q pages x l
    cols] grid, identical on every core/group.
  * A hand-assembled custom DVE op (PAGED_PAIR_SUM) consumes both phase
    streams as a [128, q, l] paged access pattern at 1 column/cycle:
    state += in0[c] + in1[c] in fp32, the state RESETS at every page
    (= run) boundary via a 1-cycle reseed bubble on SUB_DIM_DONE, and
    only the page totals are written (out_last_subdim_enable) -- the op
    emits the compacted per-segment sums [128, q] fp16 directly.  No
    masks, no prefix-sum buffer, no extraction pass, no collectives.
  * Host does only routing (sort, bucket, deal, gather); every add that
    touches row data happens on the device.

Everything data-dependent (bucket geometry, window cuts) is baked into the
compiled graph; build_bass() therefore runs after prepare().
"""

import copy
import sys
import types

import ml_dtypes
import numpy as np

if "/opt/trn_rl_repo" not in sys.path:
    sys.path.insert(0, "/opt/trn_rl_repo")

import concourse.bacc as bacc
import concourse.dve_ops as dve_ops
import concourse.mybir as mybir
import concourse.tile as tile
from concourse.dve_spec import C0, Spec, Src0, Src1
from concourse.dve_spec import AluOp as DveAluOp
from concourse.dve_spec import lower
from concourse.dve_spec import scan as dve_scan
from concourse.dve_uop import DveOpSpec, Trigger

# ----------------------------------------------------------------------------
# problem constants (hardcoded per spec)
N_ROWS = 1_000_000
NUM_CAT = 16
NUM_NUMERICS = 128
N_ARY = 32
NUM_SEGMENTS = 100_000

NCORES = 8
GROUPS = 4                    # partition-groups per core (32 feats each)
NSTREAM = NCORES * GROUPS     # 32 independent streams
PH = 2                        # phase planes (the op adds both per cycle)
NWIN = 4                      # pipeline windows per core

BF16 = ml_dtypes.bfloat16
F16 = np.float16

_dt = mybir.dt

_CACHE: dict = {}


def _paged_ref(in0, in1, s0, s1, imm2):
    x = in0.astype(np.float32) + in1.astype(np.float32)
    return x.sum(axis=-1) + np.asarray(s0, dtype=np.float32)


def _register_paged_pair_sum():
    """Custom DVE op: out[p, j] = s0 + sum_c (in0[p, j, c] + in1[p, j, c]).

    Hand-assembled 3-uop program (seed / steady / reseed) derived from the
    stock-lowered scan(ADD, Src0+Src1, init=C0):
      - steady runs the one-cycle-recurrence scan at 1 column/cycle and
        writes ONLY each page's last value (out_last_subdim_enable);
      - at every SUB_DIM_DONE (page boundary of the [P, q, l] access
        pattern) a 1-cycle non-consuming reseed bubble re-primes the
        recurrence register with C0, exactly like the initial seed.
    The per-NEFF uop table ships via the ant.dve_table HLO
    frontend-attribute path; the compile cache is pre-seeded so the DSL
    lowering (which cannot express per-page resets) is bypassed."""
    name = "PAGED_PAIR_SUM_AGG"
    for op in dve_ops.OPS:
        if op.name == name:
            return op
    spec = Spec(
        body=dve_scan(DveAluOp.ADD, Src0 + Src1, init=C0),
        reference=_paged_ref,
    )
    row = max(dve_ops._SUB_OPCODE_FOR_NAME.values()) + 1
    op = dve_ops.DveOp(name, spec, subdim=True, uops_sha={})
    dve_ops.OPS.append(op)
    dve_ops.CUSTOM_DVE_SPECS[name] = spec
    dve_ops._SUB_OPCODE_FOR_NAME[name] = row

    seed, steady = lower(
        Spec(body=dve_scan(DveAluOp.ADD, Src0 + Src1, init=C0)), ver="v3")
    seed2 = copy.deepcopy(seed)
    steady2 = copy.deepcopy(steady)
    steady2.trigger = (Trigger.SRC_TENSOR_DONE, Trigger.SUB_DIM_DONE,
                      Trigger.NONE)
    steady2.next_uop = (0, 2, 0)          # done -> IDLE, page end -> reseed
    steady2.out_last_subdim_enable = 1    # emit only page totals
    reseed = copy.deepcopy(seed)
    reseed.next_uop = (1, 0, 0)           # back to steady
    uops = [seed2, steady2, reseed]
    for ver in ("v3", "v4"):
        dve_ops._COMPILE_CACHE[(name, ver)] = DveOpSpec(
            name=name, opcode=row, uops=uops, rd1_en=True)
    return op


_PAGED = _register_paged_pair_sum()


def _ensure_axon_hooks():
    """bass_utils imports antenv.axon_hooks for trace=True; provide a shim
    so the import never fails (hook stays None unless a profiler sets it)."""
    if "antenv.axon_hooks" in sys.modules:
        return sys.modules["antenv.axon_hooks"]
    mod = types.ModuleType("antenv.axon_hooks")
    hook = [None]
    mod.set_axon_ntff_profile_hook = lambda h: hook.__setitem__(0, h)
    mod.get_axon_ntff_profile_hook = lambda: hook[0]
    sys.modules["antenv.axon_hooks"] = mod
    return mod


def _softmax64(v):
    v = np.asarray(v, dtype=np.float64)
    e = np.exp(v - v.max())
    return e / e.sum()


def prepare(inputs, idx_inputs, cat_mask, numeric_mask):
    """Host-side prep: top-k, column select + conf scale, sort, bucket by
    half run length, deal runs to 32 streams, build phase planes, cut
    page-aligned windows.

    Returns (in_maps, meta); stashes the device-graph geometry in
    _CACHE["geo"] for build_bass().
    """
    cat_mask = np.asarray(cat_mask)
    numeric_mask = np.asarray(numeric_mask)
    cm = _softmax64(cat_mask)
    ti = int(np.argmax(cm))                     # top_k(1) -> first max
    top_cat_val = cm[ti]
    nm = _softmax64(numeric_mask)
    order = np.argsort(-nm, kind="stable")[:N_ARY]   # descending, ties->low idx
    conf = ((nm[order] + top_cat_val) / 2.0).astype(np.float32)

    seg = np.ascontiguousarray(np.asarray(idx_inputs)[:, ti]).astype(np.int32)
    perm = np.argsort(seg, kind="stable")
    seg_s = seg[perm]

    inputs = np.asarray(inputs)
    sel = inputs[:, NUM_CAT + order].astype(np.float32) * conf[None, :]
    xs = sel[perm].astype(BF16)                  # [N, 32] sorted rows, bf16

    # ---- run bookkeeping ----------------------------------------------
    isstart = np.empty(N_ROWS, dtype=bool)
    isstart[0] = True
    isstart[1:] = seg_s[1:] != seg_s[:-1]
    rank_s = np.cumsum(isstart) - 1              # [N] run index of each row
    start_pos = np.flatnonzero(isstart)          # [R]
    nruns = len(start_pos)
    lens = np.empty(nruns, dtype=np.int64)
    lens[:-1] = np.diff(start_pos)
    lens[-1] = N_ROWS - start_pos[-1]
    seg_of_run = seg_s[start_pos]                # [R]
    lp = (lens + PH - 1) // PH                   # page length per run

    # ---- bucket by page length, deal to 32 streams --------------------
    # stream s <-> (core = s // GROUPS, group = s % GROUPS)
    blens = np.unique(lp)
    s_of_run = np.empty(nruns, dtype=np.int64)
    k_of_run = np.empty(nruns, dtype=np.int64)   # page index within bucket
    bkt_of_run = np.empty(nruns, dtype=np.int64)
    buckets = []                                 # (l, q, B, O) per bucket
    base = 0
    out_base = 0
    for bi, l in enumerate(blens):
        ridx = np.flatnonzero(lp == l)
        m = len(ridx)
        q = -(-m // NSTREAM)                     # pages per stream
        s_of_run[ridx] = np.arange(m) % NSTREAM
        k_of_run[ridx] = np.arange(m) // NSTREAM
        bkt_of_run[ridx] = bi
        buckets.append((int(l), int(q), int(base), int(out_base)))
        base += q * l
        out_base += q
    C2 = base
    Q = out_base

    bucket_B = np.array([b[2] for b in buckets], dtype=np.int64)
    bucket_O = np.array([b[3] for b in buckets], dtype=np.int64)
    bucket_L = np.array([b[0] for b in buckets], dtype=np.int64)
    off_of_run = bucket_B[bkt_of_run] + k_of_run * bucket_L[bkt_of_run]
    outcol_of_run = bucket_O[bkt_of_run] + k_of_run

    # ---- page-aligned window cuts and per-window op instances ---------
    bounds = [0]
    for k in range(1, NWIN):
        t = C2 * k // NWIN
        cut = C2
        for (l, q, B, O) in buckets:
            if B <= t < B + q * l:
                j = (t - B + l // 2) // l
                cut = B + j * l
                break
            if t < B:
                cut = B
                break
        bounds.append(min(cut, C2))
    bounds.append(C2)
    instances = []          # (w, local_off, n_pages, l, out_off)
    for (l, q, B, O) in buckets:
        for w in range(NWIN):
            lo = max(B, bounds[w])
            hi = min(B + q * l, bounds[w + 1])
            if lo >= hi:
                continue
            j0 = (lo - B) // l
            j1 = (hi - B) // l
            instances.append((w, lo - bounds[w], j1 - j0, l, O + j0))
    WL = max(bounds[w + 1] - bounds[w] for w in range(NWIN))

    # ---- scatter sorted rows into per-stream phase-resolved planes ----
    big = np.zeros((NSTREAM, C2 * PH, N_ARY), dtype=BF16)
    within = np.arange(N_ROWS, dtype=np.int64) - start_pos[rank_s]
    srow = s_of_run[rank_s]
    posrow = PH * off_of_run[rank_s] + within
    big.reshape(-1, N_ARY)[srow * (C2 * PH) + posrow] = xs

    # [NSTREAM, C2, PH, 32] -> [cores, 128parts, PH, C2]
    planes = big.reshape(NCORES, GROUPS, C2, PH, N_ARY)
    planes = planes.transpose(0, 1, 4, 3, 2)     # [8, g, f, p, c]
    planes = np.ascontiguousarray(planes).reshape(NCORES, 128, PH, C2)

    in_maps = []
    for i in range(NCORES):
        in_maps.append({"xin": planes[i].view(np.uint8)})

    _CACHE["geo"] = {"C2": C2, "Q": Q, "bounds": bounds,
                     "instances": instances, "WL": WL, "buckets": buckets}
    meta = {
        "seg": seg,
        "seg_of_run": seg_of_run,
        "core_of_run": s_of_run // GROUPS,
        "group_of_run": s_of_run % GROUPS,
        "outcol_of_run": outcol_of_run,
        "Q": Q,
    }
    return in_maps, meta


def build_bass():
    """Build + compile the (SPMD, per-core identical) Bass graph.

    Geometry (window cuts, paged-op instances) comes from prepare()'s
    stash, so prepare() must run first.
    """
    if "nc" in _CACHE:
        return _CACHE["nc"]
    geo = _CACHE["geo"]
    C2, Q, bounds, instances, WL = (geo["C2"], geo["Q"], geo["bounds"],
                                    geo["instances"], geo["WL"])

    nc = bacc.Bacc("TRN2", target_bir_lowering=False, debug=False,
                   num_devices=NCORES)
    xin = nc.dram_tensor("xin", [128, PH, C2 * 2], _dt.uint8,
                         kind="ExternalInput").ap()
    xout = nc.dram_tensor("out", [128, Q * 2], _dt.uint8,
                          kind="ExternalOutput").ap()

    by_win = {w: [] for w in range(NWIN)}
    for inst in instances:
        by_win[inst[0]].append(inst)

    with tile.TileContext(nc) as tc:
        with tc.tile_pool(name="pp", bufs=1) as pool, \
             tc.tile_pool(name="xp", bufs=3) as xpool:
            ot = pool.tile([128, Q], _dt.float16, tag="o")
            for w in range(NWIN):
                ws, we = bounds[w], bounds[w + 1]
                wlen = we - ws
                xt = xpool.tile([128, 2 * WL], _dt.bfloat16, tag="x",
                                name="xt")
                nc.sync.dma_start(
                    out=xt[:, 0:wlen],
                    in_=xin[:, 0, ws * 2:we * 2].bitcast(_dt.bfloat16))
                nc.scalar.dma_start(
                    out=xt[:, WL:WL + wlen],
                    in_=xin[:, 1, ws * 2:we * 2].bitcast(_dt.bfloat16))
                for (_, off, n, l, oo) in by_win[w]:
                    nc.vector._custom_dve(
                        _PAGED,
                        out=ot[:, oo:oo + n],
                        in0=xt[:, off:off + n * l].rearrange(
                            "p (q l) -> p q l", l=l),
                        in1=xt[:, WL + off:WL + off + n * l].rearrange(
                            "p (q l) -> p q l", l=l),
                        s0=0.0)
            nc.scalar.dma_start(out=xout[:], in_=ot[:].bitcast(_dt.uint8))
    nc.compile()
    _CACHE["nc"] = nc
    return nc


def postprocess(results, meta):
    """Pull per-run sums from the compacted device outputs, expand to rows."""
    table = np.zeros((NUM_SEGMENTS, N_ARY), dtype=np.float32)
    core = meta["core_of_run"]
    group = meta["group_of_run"]
    outcol = meta["outcol_of_run"]
    for i in range(NCORES):
        O = results[i]["out"].view(F16).astype(np.float32)       # [128, Q]
        O = O.reshape(GROUPS, 32, meta["Q"])
        m = core == i
        table[meta["seg_of_run"][m]] = O[group[m], :, outcol[m]]
    return table[meta["seg"]]


def run(in_maps, trace=False, trace_kwargs=None):
    _ensure_axon_hooks()
    from concourse.bass_utils import run_bass_kernel_spmd
    nc = build_bass()
    return run_bass_kernel_spmd(nc, in_maps, core_ids=list(range(NCORES)),
                                trace=trace, **(trace_kwargs or {}))


def kernel(inputs, idx_inputs, cat_mask, numeric_mask):
    in_maps, meta = prepare(inputs, idx_inputs, cat_mask, numeric_mask)
    res = run(in_maps, trace=False)
    return postprocess(res.results, meta)
